# revision 6
# baseline (speedup 1.0000x reference)
"""Trainium2 Bass kernel for nn_DGCRNN (ChebConv K=3 GNN, robot-node output).

Math: the reference returns only node 0 (robot) of the ChebConv output, so
    out = r @ (W0 - W2 + v1[0]*W1 + 2*v2[0]*W2)
        + c1 @ W1 + c2 @ (2*W2) + cheb_b
with v1 = L_hat[0, :], v2 = (L_hat @ L_hat)[0, :] (host-computed from
edge_index), c1 = sum_i v1[i] * h_i, c2 = sum_i v2[i] * h_i over the 63
human-node embeddings h_i, and r the robot embedding.

Sharding: pure data parallel over the batch dim (512 / 8 cores = 64 each);
all weights and graph data replicated.
"""

import numpy as np

B, N, F, HID = 512, 64, 64, 128
ROBOT_DIM, HUMAN_DIM = 9, 5
NCORES = 8
BL = B // NCORES      # 64 batches per core
NH = N - 1            # 63 human nodes
TOK = BL * NH         # 4032 human tokens per core
NSL = 8               # pipeline slices
BSL = BL // NSL       # 8 batches per slice
SL = BSL * NH         # 504 tokens per slice

_STATE = {}


def _build_bass():
    import concourse.bass as bass
    import concourse.tile as tile
    from concourse import bacc, mybir

    f32 = mybir.dt.float32
    AF = mybir.ActivationFunctionType
    ALU = mybir.AluOpType
    AX = mybir.AxisListType

    nc = bacc.Bacc("TRN2", target_bir_lowering=False, debug=False)

    # --- DRAM I/O (per core) ---
    d_hTa = nc.dram_tensor("hTa", [HUMAN_DIM + 1, TOK], f32, kind="ExternalInput").ap()
    d_rTa = nc.dram_tensor("rTa", [ROBOT_DIM + 1, BL], f32, kind="ExternalInput").ap()
    d_wh1a = nc.dram_tensor("wh1a", [HUMAN_DIM + 1, HID], f32, kind="ExternalInput").ap()
    d_wr1a = nc.dram_tensor("wr1a", [ROBOT_DIM + 1, HID], f32, kind="ExternalInput").ap()
    d_wh2d = nc.dram_tensor("wh2d", [HID, 2 * F], f32, kind="ExternalInput").ap()
    d_bh2d = nc.dram_tensor("bh2d", [2 * F, 1], f32, kind="ExternalInput").ap()
    d_wr2 = nc.dram_tensor("wr2", [HID, F], f32, kind="ExternalInput").ap()
    d_br2 = nc.dram_tensor("br2", [F, 1], f32, kind="ExternalInput").ap()
    d_v12 = nc.dram_tensor("v12", [2 * F, NH], f32, kind="ExternalInput").ap()
    d_W12 = nc.dram_tensor("W12", [2 * F, F], f32, kind="ExternalInput").ap()
    d_Ar = nc.dram_tensor("Ar", [F, F], f32, kind="ExternalInput").ap()
    d_ones = nc.dram_tensor("onesrow", [1, BL], f32, kind="ExternalInput").ap()
    d_chebb = nc.dram_tensor("chebb", [1, F], f32, kind="ExternalInput").ap()
    d_out = nc.dram_tensor("out", [BL, F], f32, kind="ExternalOutput").ap()

    with tile.TileContext(nc) as tc:
        with (
            tc.tile_pool(name="consts", bufs=1) as consts,
            tc.tile_pool(name="data", bufs=1) as data,
            tc.tile_pool(name="psA", bufs=2, space=bass.MemorySpace.PSUM) as psA,
            tc.tile_pool(name="psB", bufs=2, space=bass.MemorySpace.PSUM) as psB,
            tc.tile_pool(name="psO", bufs=1, space=bass.MemorySpace.PSUM) as psO,
        ):
            # constants into SBUF
            wh1a = consts.tile_from(d_wh1a, name="wh1a")
            wr1a = consts.tile_from(d_wr1a, name="wr1a")
            wh2d = consts.tile_from(d_wh2d, name="wh2d")
            bh2d = consts.tile_from(d_bh2d, name="bh2d")
            wr2 = consts.tile_from(d_wr2, name="wr2")
            br2 = consts.tile_from(d_br2, name="br2")
            v12 = consts.tile_from(d_v12, name="v12")
            W12 = consts.tile_from(d_W12, name="W12")
            Ar = consts.tile_from(d_Ar, name="Ar")
            onesr = consts.tile_from(d_ones, name="onesr")
            chebb = consts.tile_from(d_chebb, name="chebb")

            # activations
            hTa = data.tile([HUMAN_DIM + 1, TOK], f32, name="hTa_sb")
            rTa = data.tile([ROBOT_DIM + 1, BL], f32, name="rTa_sb")
            nc.sync.dma_start(hTa[:], d_hTa[:])
            nc.sync.dma_start(rTa[:], d_rTa[:])

            h1 = data.tile([HID, BL, NH], f32, name="h1_sb")
            h2d = data.tile([2 * F, BL, NH], f32, name="h2d_sb")
            tmp = data.tile([2 * F, BL, NH], f32, name="tmp_sb")
            c12 = data.tile([2 * F, BL], f32, name="c12_sb")
            r1 = data.tile([HID, BL], f32, name="r1_sb")
            r2 = data.tile([F, BL], f32, name="r2_sb")
            out_sb = data.tile([BL, F], f32, name="out_sb")

            # v12 broadcast over the batch sub-axis: [128, BSL, NH] view
            v12ap = v12[:]
            v12_b = bass.AP(
                v12ap.tensor, v12ap.offset, [list(v12ap.ap[0]), [0, BSL], list(v12ap.ap[1])]
            )

            # --- robot MLP (feature-major) ---
            pr1 = psA.tile([HID, BL], f32, name="pr1", tag="pr1", bufs=1)
            nc.tensor.matmul(pr1[:], wr1a[:], rTa[:], start=True, stop=True)
            nc.scalar.activation(r1[:], pr1[:], AF.Relu)
            pr2 = psB.tile([F, BL], f32, name="pr2", tag="pr2", bufs=1)
            nc.tensor.matmul(pr2[:], wr2[:], r1[:], start=True, stop=True)
            nc.scalar.activation(r2[:], pr2[:], AF.Relu, bias=br2[:])

            # --- human MLP, sliced pipeline ---
            for s in range(NSL):
                bsl = slice(s * BSL, (s + 1) * BSL)
                ph1 = psA.tile([HID, SL], f32, name=f"ph1_{s}", tag="ph1")
                nc.tensor.matmul(
                    ph1[:], wh1a[:], hTa[:, s * SL : (s + 1) * SL], start=True, stop=True
                )
                nc.scalar.activation(h1[:, bsl, :], ph1[:], AF.Relu)

                ph2 = psB.tile([2 * F, SL], f32, name=f"ph2_{s}", tag="ph2")
                nc.tensor.matmul(ph2[:], wh2d[:], h1[:, bsl, :], start=True, stop=True)
                nc.scalar.activation(h2d[:, bsl, :], ph2[:], AF.Relu, bias=bh2d[:])

                # weighted by [v1; v2] and segment-reduced over nodes
                nc.vector.tensor_tensor(tmp[:, bsl, :], h2d[:, bsl, :], v12_b, ALU.mult)
                nc.vector.reduce_sum(c12[:, bsl], tmp[:, bsl, :], axis=AX.X)

            # --- final combine in PSUM: out[b, fo] ---
            po = psO.tile([BL, F], f32, name="po")
            nc.tensor.matmul(po[:], c12[:], W12[:], start=True, stop=False)
            nc.tensor.matmul(po[:], r2[:], Ar[:], start=False, stop=False)
            nc.tensor.matmul(po[:], onesr[:], chebb[:], start=False, stop=True)
            nc.vector.tensor_copy(out_sb[:], po[:])
            nc.sync.dma_start(d_out[:], out_sb[:])

    nc.compile()
    return nc


def _host_prep(robot_x, human_x, edge_index, wr1_w, wr1_b, wr2_w, wr2_b,
               wh1_w, wh1_b, wh2_w, wh2_b, cheb_w, cheb_b):
    """Compute graph vectors + packed weights on host; build per-core inputs."""
    robot_x = np.ascontiguousarray(np.asarray(robot_x, dtype=np.float32))
    human_x = np.ascontiguousarray(np.asarray(human_x, dtype=np.float32))
    ei = np.asarray(edge_index)
    src, dst = ei[0].astype(np.int64), ei[1].astype(np.int64)

    f32 = np.float32
    deg = np.zeros(N, f32)
    np.add.at(deg, src, f32(1.0))
    dinv = np.where(deg > 0, deg.astype(f32) ** f32(-0.5), f32(0.0)).astype(f32)
    w = -(dinv[src] * dinv[dst])
    L = np.zeros((N, N), f32)
    np.add.at(L, (dst, src), w)
    v1 = L[0].astype(f32)
    v2 = (v1 @ L).astype(f32)

    W0, W1, W2 = (np.asarray(cheb_w, f32)[k] for k in range(3))
    wh1_w = np.asarray(wh1_w, f32); wh1_b = np.asarray(wh1_b, f32)
    wh2_w = np.asarray(wh2_w, f32); wh2_b = np.asarray(wh2_b, f32)
    wr1_w = np.asarray(wr1_w, f32); wr1_b = np.asarray(wr1_b, f32)
    wr2_w = np.asarray(wr2_w, f32); wr2_b = np.asarray(wr2_b, f32)
    cheb_b = np.asarray(cheb_b, f32)

    shared = {
        "wh1a": np.ascontiguousarray(np.vstack([wh1_w, wh1_b[None, :]])),
        "wr1a": np.ascontiguousarray(np.vstack([wr1_w, wr1_b[None, :]])),
        "wh2d": np.ascontiguousarray(np.hstack([wh2_w, wh2_w])),
        "bh2d": np.ascontiguousarray(np.concatenate([wh2_b, wh2_b])[:, None]),
        "wr2": np.ascontiguousarray(wr2_w),
        "br2": np.ascontiguousarray(wr2_b[:, None]),
        "v12": np.ascontiguousarray(
            np.vstack([np.tile(v1[1:], (F, 1)), np.tile(v2[1:], (F, 1))])
        ),
        "W12": np.ascontiguousarray(np.vstack([W1, f32(2.0) * W2])),
        "Ar": np.ascontiguousarray(
            W0 - W2 + v1[0] * W1 + f32(2.0) * v2[0] * W2
        ),
        "onesrow": np.ones((1, BL), f32),
        "chebb": np.ascontiguousarray(cheb_b[None, :]),
    }

    in_maps = []
    ones_tok = np.ones((1, TOK), f32)
    ones_bl = np.ones((1, BL), f32)
    for c in range(NCORES):
        bs = slice(c * BL, (c + 1) * BL)
        hT = human_x[bs].transpose(2, 0, 1).reshape(HUMAN_DIM, TOK)
        rT = robot_x[bs, 0, :].T
        m = dict(shared)
        m["hTa"] = np.ascontiguousarray(np.vstack([hT, ones_tok]))
        m["rTa"] = np.ascontiguousarray(np.vstack([rT, ones_bl]))
        in_maps.append(m)
    return in_maps


def run(inputs, trace=False, tmpdir=None):
    """Run the Bass kernel on 8 cores. Returns (full_output, BassKernelResults)."""
    from concourse.bass_utils import run_bass_kernel_spmd

    if "nc" not in _STATE:
        _STATE["nc"] = _build_bass()
    nc = _STATE["nc"]

    in_maps = _host_prep(**inputs)
    res = run_bass_kernel_spmd(
        nc, in_maps, list(range(NCORES)), trace=trace, tmpdir=tmpdir
    )
    out = np.concatenate([res.results[c]["out"] for c in range(NCORES)], axis=0)
    return out, res


def kernel(**inputs) -> np.ndarray:
    out, _ = run(inputs, trace=False)
    return out


# revision 20
# speedup vs baseline: 1.0785x; 1.0785x over previous
"""Trainium2 Bass kernel for nn_DGCRNN (ChebConv K=3 GNN, robot-node output).

Math: the reference returns only node 0 (robot) of the ChebConv output, so
    out = r @ (W0 - W2 + v1[0]*W1 + 2*v2[0]*W2)
        + c1 @ W1 + c2 @ (2*W2) + cheb_b
with v1 = L_hat[0, :], v2 = (L_hat @ L_hat)[0, :] (host-computed from
edge_index), c1 = sum_i v1[i] * h_i, c2 = sum_i v2[i] * h_i over the 63
human-node embeddings h_i, and r the robot embedding.

Sharding: pure data parallel over the batch dim (512 / 8 cores = 64 each);
all weights and graph data replicated.

Implementation: raw bacc (manual semaphores, no Tile) for minimal overhead.
Feature-major layout; MLP layer-1 biases folded into the matmuls via an
appended ones-row (K augmentation); wh2 duplicated along M so h2 lands twice
on 128 partitions, letting one tensor_tensor multiply by the stacked [v1;v2]
pattern and one segmented reduce produce c1,c2 stacked on 128 partitions;
the final ChebConv combine is 3 accumulating matmuls (incl. bias as a rank-1
ones x cheb_b outer product). The big MLP matmuls use float32r (TF32-rate,
single pass); everything else stays fp32.
"""

import numpy as np

B, N, F, HID = 512, 64, 64, 128
ROBOT_DIM, HUMAN_DIM = 9, 5
NCORES = 8
BL = B // NCORES      # 64 batches per core
NH = N - 1            # 63 human nodes
TOK = BL * NH         # 4032 human tokens per core
SL = 504              # tokens per matmul slice (one PSUM bank)
NPAIR = 4             # pipeline pairs; pair = 2 slices = 1008 tokens = 16 batches
PBATCH = 16           # batches per pair

_STATE = {}


def _build_bass():
    import concourse.bass as bass
    from concourse import bacc, mybir

    import os

    f32 = mybir.dt.float32
    f32r = mybir.dt.float32 if os.environ.get("DGCRNN_NO_F32R") else mybir.dt.float32r
    AF = mybir.ActivationFunctionType
    ALU = mybir.AluOpType
    AX = mybir.AxisListType

    nc = bacc.Bacc("TRN2", target_bir_lowering=False, debug=False)

    # --- DRAM I/O ---
    d_hTa = nc.dram_tensor("hTa", [HUMAN_DIM + 1, TOK], f32r, kind="ExternalInput").ap()
    d_rTa = nc.dram_tensor("rTa", [ROBOT_DIM + 1, BL], f32, kind="ExternalInput").ap()
    d_wh1a = nc.dram_tensor("wh1a", [HUMAN_DIM + 1, HID], f32r, kind="ExternalInput").ap()
    d_wr1a = nc.dram_tensor("wr1a", [ROBOT_DIM + 1, HID], f32, kind="ExternalInput").ap()
    d_wh2d = nc.dram_tensor("wh2d", [HID, 2 * F], f32r, kind="ExternalInput").ap()
    d_bh2d = nc.dram_tensor("bh2d", [2 * F, 1], f32, kind="ExternalInput").ap()
    d_wr2 = nc.dram_tensor("wr2", [HID, F], f32, kind="ExternalInput").ap()
    d_br2 = nc.dram_tensor("br2", [F, 1], f32, kind="ExternalInput").ap()
    d_v12 = nc.dram_tensor("v12", [2 * F, NH], f32, kind="ExternalInput").ap()
    d_W12 = nc.dram_tensor("W12", [2 * F, F], f32, kind="ExternalInput").ap()
    d_Ar = nc.dram_tensor("Ar", [F, F], f32, kind="ExternalInput").ap()
    d_ones = nc.dram_tensor("onesrow", [1, BL], f32, kind="ExternalInput").ap()
    d_chebb = nc.dram_tensor("chebb", [1, F], f32, kind="ExternalInput").ap()
    d_out = nc.dram_tensor("out", [BL, F], f32, kind="ExternalOutput").ap()

    # --- SBUF ---
    hTa = nc.alloc_sbuf_tensor("hTa_sb", [HUMAN_DIM + 1, TOK], f32r).ap()
    rTa = nc.alloc_sbuf_tensor("rTa_sb", [ROBOT_DIM + 1, BL], f32).ap()
    wh1a = nc.alloc_sbuf_tensor("wh1a_sb", [HUMAN_DIM + 1, HID], f32r).ap()
    wr1a = nc.alloc_sbuf_tensor("wr1a_sb", [ROBOT_DIM + 1, HID], f32).ap()
    wh2d = nc.alloc_sbuf_tensor("wh2d_sb", [HID, 2 * F], f32r).ap()
    bh2d = nc.alloc_sbuf_tensor("bh2d_sb", [2 * F, 1], f32).ap()
    wr2 = nc.alloc_sbuf_tensor("wr2_sb", [HID, F], f32).ap()
    br2 = nc.alloc_sbuf_tensor("br2_sb", [F, 1], f32).ap()
    v12 = nc.alloc_sbuf_tensor("v12_sb", [2 * F, NH], f32).ap()
    W12 = nc.alloc_sbuf_tensor("W12_sb", [2 * F, F], f32).ap()
    Ar = nc.alloc_sbuf_tensor("Ar_sb", [F, F], f32).ap()
    onesr = nc.alloc_sbuf_tensor("onesr_sb", [1, BL], f32).ap()
    chebb = nc.alloc_sbuf_tensor("chebb_sb", [1, F], f32).ap()
    h1 = nc.alloc_sbuf_tensor("h1_sb", [HID, TOK], f32r).ap()
    h2d = nc.alloc_sbuf_tensor("h2d_sb", [2 * F, BL, NH], f32).ap()
    tmp = nc.alloc_sbuf_tensor("tmp_sb", [2 * F, BL, NH], f32).ap()
    c12 = nc.alloc_sbuf_tensor("c12_sb", [2 * F, BL], f32).ap()
    r1 = nc.alloc_sbuf_tensor("r1_sb", [HID, BL], f32).ap()
    r2 = nc.alloc_sbuf_tensor("r2_sb", [F, BL], f32).ap()
    out_sb = nc.alloc_sbuf_tensor("out_sb", [BL, F], f32).ap()

    # --- PSUM (7 of 8 banks) ---
    ph1 = nc.alloc_psum_tensor("ph1", [HID, 1024], f32).ap()      # 2 banks
    ph2 = nc.alloc_psum_tensor("ph2", [2 * F, 1024], f32).ap()    # 2 banks
    pr1 = nc.alloc_psum_tensor("pr1", [HID, BL], f32).ap()        # 1 bank
    pr2 = nc.alloc_psum_tensor("pr2", [F, BL], f32).ap()          # 1 bank
    po = nc.alloc_psum_tensor("po", [BL, F], f32).ap()            # 1 bank

    # v12 broadcast over the 16-batch sub-axis of a pair
    v12_b = bass.AP(v12.tensor, v12.offset, [list(v12.ap[0]), [0, PBATCH], list(v12.ap[1])])

    # --- semaphores ---
    sdh = [nc.alloc_semaphore(f"sdh{c}") for c in range(NPAIR)]  # per hTa chunk
    sdw = nc.alloc_semaphore("sdw")    # gpsimd const DMAs
    sdr = nc.alloc_semaphore("sdr")    # scalar const DMAs
    sdf = nc.alloc_semaphore("sdf")    # final-const DMAs (sync)
    sp = nc.alloc_semaphore("sp")      # PE groups done
    sa = nc.alloc_semaphore("sa")      # ACT ops done
    sv = nc.alloc_semaphore("sv")      # DVE ops done
    sg = nc.alloc_semaphore("sg")      # GPS ops done
    sq = nc.alloc_semaphore("sq")      # out DMA done
    sdone = nc.alloc_semaphore("sdone")
    all_sems = sdh + [sdw, sdr, sdf, sp, sa, sv, sg, sq, sdone]

    PC = 1008  # tokens per pair

    def pair_h1_cols(p):
        return slice(p * PC, (p + 1) * PC)

    def pair_b(p):
        return slice(p * PBATCH, (p + 1) * PBATCH)

    with nc.Block(no_gpsimd_drain=False) as block:

        @block.sync
        def _(sync):
            for c in range(NPAIR):
                sync.dma_start(
                    out=hTa[:, c * PC : (c + 1) * PC],
                    in_=d_hTa[:, c * PC : (c + 1) * PC],
                ).then_inc(sdh[c], 16)
            sync.dma_start(out=W12[:], in_=d_W12[:]).then_inc(sdf, 16)
            sync.dma_start(out=Ar[:], in_=d_Ar[:]).then_inc(sdf, 16)
            sync.dma_start(out=onesr[:], in_=d_ones[:]).then_inc(sdf, 16)
            sync.dma_start(out=chebb[:], in_=d_chebb[:]).then_inc(sdf, 16)
            sync.wait_ge(sv, 7)
            sync.dma_start(out=d_out[:], in_=out_sb[:]).then_inc(sq, 16)
            sync.wait_ge(sq, 16)
            sync.wait_ge(sdone, 4)

        @block.tensor
        def _(tensor):
            # robot MLP (fp32)
            tensor.wait_ge(sdr, 64)                     # rTa, wr1a, wr2, br2
            tensor.matmul(pr1[:], wr1a[:], rTa[:], start=True, stop=True).then_inc(sp)
            tensor.wait_ge(sa, 1)                       # r1 relu
            tensor.matmul(pr2[:], wr2[:], r1[:], start=True, stop=True).then_inc(sp)
            # human MLP, f32r
            tensor.wait_ge(sdw, 64)                     # wh1a, wh2d, bh2d, v12
            # ACT op index bookkeeping: 1=r1relu, 2=relu1(0), 3=r2relu,
            # 4=relu1(1), 5=relu2(0), 6=relu1(2), 7=relu2(1), 8=relu1(3)
            relu1_done = [2, 4, 6, 8]
            relu2_done = {0: ("sa", 5), 1: ("sa", 7)}   # 2,3 on DVE
            for p in range(NPAIR):
                tensor.wait_ge(sdh[p], 16)
                if p >= 1:
                    tensor.wait_ge(sa, relu1_done[p - 1])  # ph1 WAR
                cols = pair_h1_cols(p)
                tensor.matmul(
                    ph1[:, 0:SL], wh1a[:], hTa[:, p * PC : p * PC + SL],
                    start=True, stop=True,
                )
                tensor.matmul(
                    ph1[:, 512 : 512 + SL], wh1a[:], hTa[:, p * PC + SL : (p + 1) * PC],
                    start=True, stop=True,
                ).then_inc(sp)                           # sp = 3 + 2p
                # L2 for this pair
                tensor.wait_ge(sa, relu1_done[p])        # h1 pair ready
                if p >= 1:
                    # ph2 WAR: previous pair's relu2 consumed it
                    eng, val = relu2_done.get(p - 1, (None, None))
                    if eng == "sa":
                        tensor.wait_ge(sa, val)
                    else:
                        tensor.wait_ge(sv, 1 if p - 1 == 2 else 4)
                tensor.matmul(
                    ph2[:, 0:SL], wh2d[:], h1[:, p * PC : p * PC + SL],
                    start=True, stop=True,
                )
                tensor.matmul(
                    ph2[:, 512 : 512 + SL], wh2d[:], h1[:, p * PC + SL : (p + 1) * PC],
                    start=True, stop=True,
                ).then_inc(sp)                           # sp = 4 + 2p
            # final combine (fp32): out = c12.T@W12 + r2.T@Ar + ones x chebb
            tensor.wait_ge(sv, 6)                        # c12 complete
            tensor.wait_ge(sdf, 64)                      # W12, Ar, onesr, chebb
            tensor.matmul(po[:], c12[:], W12[:], start=True, stop=False)
            tensor.matmul(po[:], r2[:], Ar[:], start=False, stop=False)
            tensor.matmul(po[:], onesr[:], chebb[:], start=False, stop=True).then_inc(sp)  # sp=11
            tensor.sem_inc(sdone, 1)

        @block.scalar
        def _(scalar):
            # const DMAs first (robot path + L2 bias)
            scalar.dma_start(out=rTa[:], in_=d_rTa[:]).then_inc(sdr, 16)
            scalar.dma_start(out=wr1a[:], in_=d_wr1a[:]).then_inc(sdr, 16)
            scalar.dma_start(out=wr2[:], in_=d_wr2[:]).then_inc(sdr, 16)
            scalar.dma_start(out=br2[:], in_=d_br2[:]).then_inc(sdr, 16)
            # ACT#1: robot layer-1 relu (bias folded in K-aug)
            scalar.wait_ge(sp, 1)
            scalar.activation(r1[:], pr1[:], AF.Relu).then_inc(sa)
            # ACT#2: relu1(0)
            scalar.wait_ge(sp, 3)
            scalar.activation(
                h1[:, pair_h1_cols(0)],
                bass.AP(ph1.tensor, ph1.offset, [list(ph1.ap[0]), [512, 2], [1, SL]]),
                AF.Relu,
            ).then_inc(sa)
            # ACT#3: robot layer-2 relu (+bias)
            scalar.wait_ge(sp, 2)
            scalar.activation(r2[:], pr2[:], AF.Relu, bias=br2[:]).then_inc(sa)
            # ACT#4: relu1(1)
            scalar.wait_ge(sp, 5)
            scalar.activation(
                h1[:, pair_h1_cols(1)],
                bass.AP(ph1.tensor, ph1.offset, [list(ph1.ap[0]), [512, 2], [1, SL]]),
                AF.Relu,
            ).then_inc(sa)
            # ACT#5: relu2(0) (+bias)
            scalar.wait_ge(sp, 4)
            scalar.activation(
                h2d[:, pair_b(0), :],
                bass.AP(ph2.tensor, ph2.offset, [list(ph2.ap[0]), [512, 2], [1, SL]]),
                AF.Relu,
                bias=bh2d[:],
            ).then_inc(sa)
            # ACT#6: relu1(2)
            scalar.wait_ge(sp, 7)
            scalar.activation(
                h1[:, pair_h1_cols(2)],
                bass.AP(ph1.tensor, ph1.offset, [list(ph1.ap[0]), [512, 2], [1, SL]]),
                AF.Relu,
            ).then_inc(sa)
            # ACT#7: relu2(1) (+bias)
            scalar.wait_ge(sp, 6)
            scalar.activation(
                h2d[:, pair_b(1), :],
                bass.AP(ph2.tensor, ph2.offset, [list(ph2.ap[0]), [512, 2], [1, SL]]),
                AF.Relu,
                bias=bh2d[:],
            ).then_inc(sa)
            # ACT#8: relu1(3)
            scalar.wait_ge(sp, 9)
            scalar.activation(
                h1[:, pair_h1_cols(3)],
                bass.AP(ph1.tensor, ph1.offset, [list(ph1.ap[0]), [512, 2], [1, SL]]),
                AF.Relu,
            ).then_inc(sa)
            scalar.sem_inc(sdone, 1)

        @block.vector
        def _(vector):
            # DVE#1: relu2(2) via tensor_scalar (bias-add then max-0)
            vector.wait_ge(sp, 8)
            vector.tensor_scalar(
                h2d[:, pair_b(2), :],
                bass.AP(ph2.tensor, ph2.offset, [list(ph2.ap[0]), [512, 2], [1, SL]]),
                bh2d[:],
                0.0,
                op0=ALU.add,
                op1=ALU.max,
            ).then_inc(sv)
            # DVE#2: mul(0)
            vector.wait_ge(sa, 5)
            vector.wait_ge(sdw, 64)  # v12
            vector.tensor_tensor(
                tmp[:, pair_b(0), :], h2d[:, pair_b(0), :], v12_b, ALU.mult
            ).then_inc(sv)
            # DVE#3: mul(1)
            vector.wait_ge(sa, 7)
            vector.tensor_tensor(
                tmp[:, pair_b(1), :], h2d[:, pair_b(1), :], v12_b, ALU.mult
            ).then_inc(sv)
            # DVE#4: relu2(3)
            vector.wait_ge(sp, 10)
            vector.tensor_scalar(
                h2d[:, pair_b(3), :],
                bass.AP(ph2.tensor, ph2.offset, [list(ph2.ap[0]), [512, 2], [1, SL]]),
                bh2d[:],
                0.0,
                op0=ALU.add,
                op1=ALU.max,
            ).then_inc(sv)
            # DVE#5: red(0) over batches 0..31 (muls 0,1 are DVE #2,#3)
            vector.wait_ge(sv, 3)
            vector.tensor_reduce(
                c12[:, 0:32], tmp[:, 0:32, :], axis=AX.X, op=ALU.add
            ).then_inc(sv)
            # DVE#6: red(1) over batches 32..63 (needs GPS muls 2,3)
            vector.wait_ge(sg, 2)
            vector.tensor_reduce(
                c12[:, 32:64], tmp[:, 32:64, :], axis=AX.X, op=ALU.add
            ).then_inc(sv)
            # DVE#7: final copy PSUM -> SBUF
            vector.wait_ge(sp, 11)
            vector.tensor_copy(out_sb[:], po[:]).then_inc(sv)
            vector.sem_inc(sdone, 1)

        @block.gpsimd
        def _(gpsimd):
            gpsimd.dma_start(out=wh1a[:], in_=d_wh1a[:]).then_inc(sdw, 16)
            gpsimd.dma_start(out=wh2d[:], in_=d_wh2d[:]).then_inc(sdw, 16)
            gpsimd.dma_start(out=bh2d[:], in_=d_bh2d[:]).then_inc(sdw, 16)
            gpsimd.dma_start(out=v12[:], in_=d_v12[:]).then_inc(sdw, 16)
            # GPS#1: mul(2)
            gpsimd.wait_ge(sv, 1)
            gpsimd.tensor_tensor(
                tmp[:, pair_b(2), :], h2d[:, pair_b(2), :], v12_b, ALU.mult
            ).then_inc(sg)
            # GPS#2: mul(3)
            gpsimd.wait_ge(sv, 4)
            gpsimd.tensor_tensor(
                tmp[:, pair_b(3), :], h2d[:, pair_b(3), :], v12_b, ALU.mult
            ).then_inc(sg)
            gpsimd.sem_inc(sdone, 1)

    # sems must return to 0 for NEFF re-execution; the Block exit emitted an
    # all-engine barrier, so clearing here is safe.
    nc.clear_and_free_semaphores(all_sems)

    nc.compile()
    return nc


def _host_prep(robot_x, human_x, edge_index, wr1_w, wr1_b, wr2_w, wr2_b,
               wh1_w, wh1_b, wh2_w, wh2_b, cheb_w, cheb_b):
    """Compute graph vectors + packed weights on host; build per-core inputs."""
    robot_x = np.ascontiguousarray(np.asarray(robot_x, dtype=np.float32))
    human_x = np.ascontiguousarray(np.asarray(human_x, dtype=np.float32))
    ei = np.asarray(edge_index)
    src, dst = ei[0].astype(np.int64), ei[1].astype(np.int64)

    f32 = np.float32
    deg = np.zeros(N, f32)
    np.add.at(deg, src, f32(1.0))
    dinv = np.where(deg > 0, deg.astype(f32) ** f32(-0.5), f32(0.0)).astype(f32)
    w = -(dinv[src] * dinv[dst])
    L = np.zeros((N, N), f32)
    np.add.at(L, (dst, src), w)
    v1 = L[0].astype(f32)
    v2 = (v1 @ L).astype(f32)

    W0, W1, W2 = (np.asarray(cheb_w, f32)[k] for k in range(3))
    wh1_w = np.asarray(wh1_w, f32); wh1_b = np.asarray(wh1_b, f32)
    wh2_w = np.asarray(wh2_w, f32); wh2_b = np.asarray(wh2_b, f32)
    wr1_w = np.asarray(wr1_w, f32); wr1_b = np.asarray(wr1_b, f32)
    wr2_w = np.asarray(wr2_w, f32); wr2_b = np.asarray(wr2_b, f32)
    cheb_b = np.asarray(cheb_b, f32)

    shared = {
        "wh1a": np.ascontiguousarray(np.vstack([wh1_w, wh1_b[None, :]])),
        "wr1a": np.ascontiguousarray(np.vstack([wr1_w, wr1_b[None, :]])),
        "wh2d": np.ascontiguousarray(np.hstack([wh2_w, wh2_w])),
        "bh2d": np.ascontiguousarray(np.concatenate([wh2_b, wh2_b])[:, None]),
        "wr2": np.ascontiguousarray(wr2_w),
        "br2": np.ascontiguousarray(wr2_b[:, None]),
        "v12": np.ascontiguousarray(
            np.vstack([np.tile(v1[1:], (F, 1)), np.tile(v2[1:], (F, 1))])
        ),
        "W12": np.ascontiguousarray(np.vstack([W1, f32(2.0) * W2])),
        "Ar": np.ascontiguousarray(
            W0 - W2 + v1[0] * W1 + f32(2.0) * v2[0] * W2
        ),
        "onesrow": np.ones((1, BL), f32),
        "chebb": np.ascontiguousarray(cheb_b[None, :]),
    }

    in_maps = []
    ones_tok = np.ones((1, TOK), f32)
    ones_bl = np.ones((1, BL), f32)
    for c in range(NCORES):
        bs = slice(c * BL, (c + 1) * BL)
        hT = human_x[bs].transpose(2, 0, 1).reshape(HUMAN_DIM, TOK)
        rT = robot_x[bs, 0, :].T
        m = dict(shared)
        m["hTa"] = np.ascontiguousarray(np.vstack([hT, ones_tok]))
        m["rTa"] = np.ascontiguousarray(np.vstack([rT, ones_bl]))
        in_maps.append(m)
    return in_maps


def run(inputs, trace=False, tmpdir=None):
    """Run the Bass kernel on 8 cores. Returns (full_output, BassKernelResults)."""
    from concourse.bass_utils import run_bass_kernel_spmd

    if "nc" not in _STATE:
        _STATE["nc"] = _build_bass()
    nc = _STATE["nc"]

    in_maps = _host_prep(**inputs)
    res = run_bass_kernel_spmd(
        nc, in_maps, list(range(NCORES)), trace=trace, tmpdir=tmpdir
    )
    out = np.concatenate([res.results[c]["out"] for c in range(NCORES)], axis=0)
    return out, res


def kernel(**inputs) -> np.ndarray:
    out, _ = run(inputs, trace=False)
    return out


# revision 21
# speedup vs baseline: 1.2467x; 1.1560x over previous
"""Trainium2 Bass kernel for nn_DGCRNN (ChebConv K=3 GNN, robot-node output).

Math: the reference returns only node 0 (robot) of the ChebConv output, so
    out = r @ (W0 - W2 + v1[0]*W1 + 2*v2[0]*W2)
        + c1 @ W1 + c2 @ (2*W2) + cheb_b
with v1 = L_hat[0, :], v2 = (L_hat @ L_hat)[0, :] (host-computed from
edge_index), c1 = sum_i v1[i] * h_i, c2 = sum_i v2[i] * h_i over the 63
human-node embeddings h_i, and r the robot embedding.

Sharding: pure data parallel over the batch dim (512 / 8 cores = 64 each);
all weights and graph data replicated.

Implementation: raw bacc (manual semaphores, no Tile) for minimal overhead.
Feature-major layout; MLP layer-1 biases folded into the matmuls via an
appended ones-row (K augmentation); wh2 duplicated along M so h2 lands twice
on 128 partitions, letting one tensor_tensor multiply by the stacked [v1;v2]
pattern and one segmented reduce produce c1,c2 stacked on 128 partitions;
the final ChebConv combine is 3 accumulating matmuls (incl. bias as a rank-1
ones x cheb_b outer product). The big MLP matmuls use float32r (TF32-rate,
single pass); everything else stays fp32.
"""

import numpy as np

B, N, F, HID = 512, 64, 64, 128
ROBOT_DIM, HUMAN_DIM = 9, 5
NCORES = 8
BL = B // NCORES      # 64 batches per core
NH = N - 1            # 63 human nodes
TOK = BL * NH         # 4032 human tokens per core
SL = 504              # tokens per matmul slice (one PSUM bank)
NPAIR = 4             # pipeline pairs; pair = 2 slices = 1008 tokens = 16 batches
PBATCH = 16           # batches per pair

_STATE = {}


def _build_bass():
    import os

    import concourse.bass as bass
    from concourse import bacc, mybir

    f32 = mybir.dt.float32
    f32r = mybir.dt.float32 if os.environ.get("DGCRNN_NO_F32R") else mybir.dt.float32r
    AF = mybir.ActivationFunctionType
    ALU = mybir.AluOpType
    AX = mybir.AxisListType

    nc = bacc.Bacc("TRN2", target_bir_lowering=False, debug=False)

    # --- DRAM I/O ---
    d_hTa = nc.dram_tensor("hTa", [HUMAN_DIM + 1, TOK], f32r, kind="ExternalInput").ap()
    d_p32 = nc.dram_tensor("p32", [HID, 577], f32, kind="ExternalInput").ap()
    d_pr = nc.dram_tensor("pr", [HID, 256], f32r, kind="ExternalInput").ap()
    d_out = nc.dram_tensor("out", [BL, F], f32, kind="ExternalOutput").ap()

    # --- SBUF ---
    hTa = nc.alloc_sbuf_tensor("hTa_sb", [HUMAN_DIM + 1, TOK], f32r).ap()
    p32 = nc.alloc_sbuf_tensor("p32_sb", [HID, 577], f32).ap()
    pr_ = nc.alloc_sbuf_tensor("pr_sb", [HID, 256], f32r).ap()
    h1 = nc.alloc_sbuf_tensor("h1_sb", [HID, TOK], f32r).ap()
    h2d = nc.alloc_sbuf_tensor("h2d_sb", [2 * F, BL, NH], f32).ap()
    tmp = nc.alloc_sbuf_tensor("tmp_sb", [2 * F, BL, NH], f32).ap()
    c12 = nc.alloc_sbuf_tensor("c12_sb", [2 * F, BL], f32).ap()
    r1 = nc.alloc_sbuf_tensor("r1_sb", [HID, BL], f32).ap()
    r2 = nc.alloc_sbuf_tensor("r2_sb", [F, BL], f32).ap()
    out_sb = nc.alloc_sbuf_tensor("out_sb", [BL, F], f32).ap()

    # pack32 slices
    wr2 = p32[:, 0:64]
    W12 = p32[:, 64:128]
    v12 = p32[:, 128:191]
    bh2d = p32[:, 191:192]
    Ar = p32[0:64, 192:256]
    br2 = p32[0:64, 256:257]
    rTa = p32[0:ROBOT_DIM + 1, 257:321]
    wr1a = p32[0:ROBOT_DIM + 1, 321:449]
    onesr = p32[0:1, 449:513]
    chebb = p32[0:1, 513:577]
    # packr slices (f32r)
    wh2d = pr_[:, 0:128]
    wh1a = pr_[0:HUMAN_DIM + 1, 128:256]

    # --- PSUM: ping-pong, 8 banks total ---
    ph1 = nc.alloc_psum_tensor("ph1", [HID, 2048], f32).ap()      # 4 banks
    ph2 = nc.alloc_psum_tensor("ph2", [2 * F, 2048], f32).ap()    # 4 banks
    pr1 = ph1[:, 0:BL]          # robot L1 out, freed by r1relu before L1(0)
    pr2 = ph2[:F, 0:BL]         # robot L2 out, freed by r2relu before L2(0)
    po = ph2[:BL, 0:F]          # final out, after relu2(2) consumed ping

    v12_b = bass.AP(v12.tensor, v12.offset, [list(v12.ap[0]), [0, PBATCH], [1, NH]])

    # --- semaphores ---
    sdh = [nc.alloc_semaphore(f"sdh{c}") for c in range(NPAIR)]  # per hTa chunk
    sdr = nc.alloc_semaphore("sdr")    # pack32 DMA
    sdw = nc.alloc_semaphore("sdw")    # packr DMA
    sp = nc.alloc_semaphore("sp")      # PE groups done
    sa = nc.alloc_semaphore("sa")      # ACT ops done
    sv = nc.alloc_semaphore("sv")      # DVE ops done
    sg = nc.alloc_semaphore("sg")      # GPS ops done
    sq = nc.alloc_semaphore("sq")      # out DMA done
    sdone = nc.alloc_semaphore("sdone")
    all_sems = sdh + [sdr, sdw, sp, sa, sv, sg, sq, sdone]

    PC = 1008  # tokens per pair

    def ping(p):
        return (p % 2) * 1024

    def pair_b(p):
        return slice(p * PBATCH, (p + 1) * PBATCH)

    def ph_in(ph, p):
        o = ping(p)
        return bass.AP(ph.tensor, ph.offset + o, [list(ph.ap[0]), [512, 2], [1, SL]])

    with nc.Block(no_gpsimd_drain=True) as block:

        @block.sync
        def _(sync):
            for c in range(NPAIR):
                sync.dma_start(
                    out=hTa[:, c * PC : (c + 1) * PC],
                    in_=d_hTa[:, c * PC : (c + 1) * PC],
                ).then_inc(sdh[c], 16)
            sync.wait_ge(sv, 7)
            sync.dma_start(out=d_out[:], in_=out_sb[:]).then_inc(sq, 16)
            sync.wait_ge(sq, 16)
            sync.wait_ge(sdone, 4)

        @block.tensor
        def _(tensor):
            # sp: 1=rMM1 2=rMM2 3=L1(0) 4=L1(1) 5=L2(0) 6=L1(2) 7=L2(1)
            #     8=L1(3) 9=L2(2) 10=L2(3) 11=finals
            def l1(p, *waits):
                for s, v in waits:
                    tensor.wait_ge(s, v)
                o = ping(p)
                tensor.matmul(ph1[:, o : o + SL], wh1a, hTa[:, p * PC : p * PC + SL],
                              start=True, stop=True)
                tensor.matmul(ph1[:, o + 512 : o + 512 + SL], wh1a,
                              hTa[:, p * PC + SL : (p + 1) * PC],
                              start=True, stop=True).then_inc(sp)

            def l2(p, *waits):
                for s, v in waits:
                    tensor.wait_ge(s, v)
                o = ping(p)
                tensor.matmul(ph2[:, o : o + SL], wh2d, h1[:, p * PC : p * PC + SL],
                              start=True, stop=True)
                tensor.matmul(ph2[:, o + 512 : o + 512 + SL], wh2d,
                              h1[:, p * PC + SL : (p + 1) * PC],
                              start=True, stop=True).then_inc(sp)

            tensor.wait_ge(sdr, 16)
            tensor.matmul(pr1, wr1a, rTa, start=True, stop=True).then_inc(sp)   # 1
            tensor.wait_ge(sa, 1)
            tensor.matmul(pr2, wr2, r1[:], start=True, stop=True).then_inc(sp)  # 2
            tensor.wait_ge(sdw, 16)
            l1(0, (sdh[0], 16), (sa, 1))            # 3 (pr1 region WAR)
            l1(1, (sdh[1], 16))                     # 4
            l2(0, (sa, 3))                          # 5 (relu1(0) + r2relu WAR)
            l1(2, (sdh[2], 16), (sa, 3))            # 6 (ping freed)
            l2(1, (sa, 4))                          # 7
            l1(3, (sdh[3], 16), (sa, 4))            # 8
            l2(2, (sa, 6))                          # 9 (ping WAR via relu2(0)<=6)
            l2(3, (sa, 8))                          # 10
            tensor.wait_ge(sv, 6)                   # c12 ready; po region free
            tensor.matmul(po, c12[:], W12, start=True, stop=False)
            tensor.matmul(po, r2[:], Ar, start=False, stop=False)
            tensor.matmul(po, onesr, chebb, start=False, stop=True).then_inc(sp)  # 11
            tensor.sem_inc(sdone, 1)

        @block.scalar
        def _(scalar):
            scalar.dma_start(out=p32[:], in_=d_p32[:]).then_inc(sdr, 16)
            scalar.dma_start(out=pr_[:], in_=d_pr[:]).then_inc(sdw, 16)
            # sa: 1=r1relu 2=r2relu 3=relu1(0) 4=relu1(1) 5=relu2(0)
            #     6=relu1(2) 7=relu2(1) 8=relu1(3)
            scalar.wait_ge(sp, 1)
            scalar.activation(r1[:], pr1, AF.Relu).then_inc(sa)
            scalar.wait_ge(sp, 2)
            scalar.activation(r2[:], pr2, AF.Relu, bias=br2).then_inc(sa)
            scalar.wait_ge(sp, 3)
            scalar.activation(h1[:, 0:PC], ph_in(ph1, 0), AF.Relu).then_inc(sa)
            scalar.wait_ge(sp, 4)
            scalar.activation(h1[:, PC : 2 * PC], ph_in(ph1, 1), AF.Relu).then_inc(sa)
            scalar.wait_ge(sp, 5)
            scalar.activation(h2d[:, pair_b(0), :], ph_in(ph2, 0), AF.Relu,
                              bias=bh2d).then_inc(sa)
            scalar.wait_ge(sp, 6)
            scalar.activation(h1[:, 2 * PC : 3 * PC], ph_in(ph1, 2), AF.Relu).then_inc(sa)
            scalar.wait_ge(sp, 7)
            scalar.activation(h2d[:, pair_b(1), :], ph_in(ph2, 1), AF.Relu,
                              bias=bh2d).then_inc(sa)
            scalar.wait_ge(sp, 8)
            scalar.activation(h1[:, 3 * PC : 4 * PC], ph_in(ph1, 3), AF.Relu).then_inc(sa)
            scalar.sem_inc(sdone, 1)

        @block.vector
        def _(vector):
            # sv: 1=relu2(2) 2=mul(0) 3=mul(1) 4=relu2(3) 5=red(0) 6=red(1) 7=copy
            vector.wait_ge(sp, 9)
            vector.tensor_scalar(h2d[:, pair_b(2), :], ph_in(ph2, 2), bh2d, 0.0,
                                 op0=ALU.add, op1=ALU.max).then_inc(sv)
            vector.wait_ge(sa, 5)
            vector.wait_ge(sdr, 16)
            vector.tensor_tensor(tmp[:, pair_b(0), :], h2d[:, pair_b(0), :],
                                 v12_b, ALU.mult).then_inc(sv)
            vector.wait_ge(sa, 7)
            vector.tensor_tensor(tmp[:, pair_b(1), :], h2d[:, pair_b(1), :],
                                 v12_b, ALU.mult).then_inc(sv)
            vector.wait_ge(sp, 10)
            vector.tensor_scalar(h2d[:, pair_b(3), :], ph_in(ph2, 3), bh2d, 0.0,
                                 op0=ALU.add, op1=ALU.max).then_inc(sv)
            vector.wait_ge(sv, 3)
            vector.tensor_reduce(c12[:, 0:32], tmp[:, 0:32, :], axis=AX.X,
                                 op=ALU.add).then_inc(sv)
            vector.wait_ge(sg, 2)
            vector.tensor_reduce(c12[:, 32:64], tmp[:, 32:64, :], axis=AX.X,
                                 op=ALU.add).then_inc(sv)
            vector.wait_ge(sp, 11)
            vector.tensor_copy(out_sb[:], po).then_inc(sv)
            vector.sem_inc(sdone, 1)

        @block.gpsimd
        def _(gpsimd):
            gpsimd.wait_ge(sv, 1)
            gpsimd.tensor_tensor(tmp[:, pair_b(2), :], h2d[:, pair_b(2), :],
                                 v12_b, ALU.mult).then_inc(sg)
            gpsimd.wait_ge(sv, 4)
            gpsimd.tensor_tensor(tmp[:, pair_b(3), :], h2d[:, pair_b(3), :],
                                 v12_b, ALU.mult).then_inc(sg)
            gpsimd.sem_inc(sdone, 1)

    # sems must return to 0 for NEFF re-execution; the Block exit emitted an
    # all-engine barrier, so clearing here is safe.
    nc.clear_and_free_semaphores(all_sems)

    nc.compile()
    return nc


def _host_prep(robot_x, human_x, edge_index, wr1_w, wr1_b, wr2_w, wr2_b,
               wh1_w, wh1_b, wh2_w, wh2_b, cheb_w, cheb_b):
    """Compute graph vectors + packed weights on host; build per-core inputs."""
    robot_x = np.ascontiguousarray(np.asarray(robot_x, dtype=np.float32))
    human_x = np.ascontiguousarray(np.asarray(human_x, dtype=np.float32))
    ei = np.asarray(edge_index)
    src, dst = ei[0].astype(np.int64), ei[1].astype(np.int64)

    f32 = np.float32
    deg = np.zeros(N, f32)
    np.add.at(deg, src, f32(1.0))
    dinv = np.where(deg > 0, deg.astype(f32) ** f32(-0.5), f32(0.0)).astype(f32)
    w = -(dinv[src] * dinv[dst])
    L = np.zeros((N, N), f32)
    np.add.at(L, (dst, src), w)
    v1 = L[0].astype(f32)
    v2 = (v1 @ L).astype(f32)

    W0, W1, W2 = (np.asarray(cheb_w, f32)[k] for k in range(3))
    wh1_w = np.asarray(wh1_w, f32); wh1_b = np.asarray(wh1_b, f32)
    wh2_w = np.asarray(wh2_w, f32); wh2_b = np.asarray(wh2_b, f32)
    wr1_w = np.asarray(wr1_w, f32); wr1_b = np.asarray(wr1_b, f32)
    wr2_w = np.asarray(wr2_w, f32); wr2_b = np.asarray(wr2_b, f32)
    cheb_b = np.asarray(cheb_b, f32)

    p32 = np.zeros((HID, 577), f32)
    p32[:, 0:64] = wr2_w
    p32[0:64, 64:128] = W1
    p32[64:128, 64:128] = f32(2.0) * W2
    p32[0:64, 128:191] = np.tile(v1[1:], (F, 1))
    p32[64:128, 128:191] = np.tile(v2[1:], (F, 1))
    p32[0:64, 191] = wh2_b
    p32[64:128, 191] = wh2_b
    p32[0:64, 192:256] = W0 - W2 + v1[0] * W1 + f32(2.0) * v2[0] * W2
    p32[0:64, 256] = wr2_b
    p32[0:ROBOT_DIM, 321:449] = wr1_w
    p32[ROBOT_DIM, 321:449] = wr1_b
    p32[0, 449:513] = f32(1.0)
    p32[0, 513:577] = cheb_b
    pr = np.zeros((HID, 256), f32)
    pr[:, 0:128] = np.hstack([wh2_w, wh2_w])
    pr[0:HUMAN_DIM, 128:256] = wh1_w
    pr[HUMAN_DIM, 128:256] = wh1_b
    shared = {"p32": p32, "pr": pr}

    in_maps = []
    ones_tok = np.ones((1, TOK), f32)
    for c in range(NCORES):
        bs = slice(c * BL, (c + 1) * BL)
        hT = human_x[bs].transpose(2, 0, 1).reshape(HUMAN_DIM, TOK)
        m = dict(shared)
        m["hTa"] = np.ascontiguousarray(np.vstack([hT, ones_tok]))
        p32c = shared["p32"].copy()
        p32c[0:ROBOT_DIM, 257:321] = robot_x[bs, 0, :].T
        p32c[ROBOT_DIM, 257:321] = f32(1.0)
        m["p32"] = p32c
        in_maps.append(m)
    return in_maps


def run(inputs, trace=False, tmpdir=None):
    """Run the Bass kernel on 8 cores. Returns (full_output, BassKernelResults)."""
    from concourse.bass_utils import run_bass_kernel_spmd

    if "nc" not in _STATE:
        _STATE["nc"] = _build_bass()
    nc = _STATE["nc"]

    in_maps = _host_prep(**inputs)
    res = run_bass_kernel_spmd(
        nc, in_maps, list(range(NCORES)), trace=trace, tmpdir=tmpdir
    )
    out = np.concatenate([res.results[c]["out"] for c in range(NCORES)], axis=0)
    return out, res


def kernel(**inputs) -> np.ndarray:
    out, _ = run(inputs, trace=False)
    return out


# revision 22
# speedup vs baseline: 1.3970x; 1.1206x over previous
"""Trainium2 Bass kernel for nn_DGCRNN (ChebConv K=3 GNN, robot-node output).

Math: the reference returns only node 0 (robot) of the ChebConv output, so
    out = r @ (W0 - W2 + v1[0]*W1 + 2*v2[0]*W2)
        + c1 @ W1 + c2 @ (2*W2) + cheb_b
with v1 = L_hat[0, :], v2 = (L_hat @ L_hat)[0, :] (host-computed from
edge_index), c1 = sum_i v1[i] * h_i, c2 = sum_i v2[i] * h_i over the 63
human-node embeddings h_i, and r the robot embedding.

Sharding: pure data parallel over the batch dim (512 / 8 cores = 64 each);
all weights and graph data replicated.

Implementation: raw bacc (manual semaphores, no Tile) for minimal overhead.
Feature-major layout; MLP layer-1 biases folded into the matmuls via an
appended ones-row (K augmentation); wh2 duplicated along M so h2 lands twice
on 128 partitions, letting one tensor_tensor multiply by the stacked [v1;v2]
pattern and one segmented reduce produce c1,c2 stacked on 128 partitions;
the final ChebConv combine is 3 accumulating matmuls (incl. bias as a rank-1
ones x cheb_b outer product). The big MLP matmuls use float32r (TF32-rate,
single pass); everything else stays fp32.
"""

import numpy as np

B, N, F, HID = 512, 64, 64, 128
ROBOT_DIM, HUMAN_DIM = 9, 5
NCORES = 8
BL = B // NCORES      # 64 batches per core
NH = N - 1            # 63 human nodes
TOK = BL * NH         # 4032 human tokens per core
SL = 504              # tokens per matmul slice (one PSUM bank)
NPAIR = 4             # pipeline pairs; pair = 2 slices = 1008 tokens = 16 batches
PBATCH = 16           # batches per pair

_STATE = {}


def _build_bass():
    import os

    import concourse.bass as bass
    from concourse import bacc, mybir

    f32 = mybir.dt.float32
    f32r = mybir.dt.float32 if os.environ.get("DGCRNN_NO_F32R") else mybir.dt.float32r
    AF = mybir.ActivationFunctionType
    ALU = mybir.AluOpType
    AX = mybir.AxisListType

    nc = bacc.Bacc("TRN2", target_bir_lowering=False, debug=False)

    # --- DRAM I/O ---
    d_hTa = nc.dram_tensor("hTa", [HUMAN_DIM + 1, TOK], f32r, kind="ExternalInput").ap()
    d_pa = nc.dram_tensor("pa", [HID, 257], f32, kind="ExternalInput").ap()
    d_pb = nc.dram_tensor("pb", [HID, 320], f32, kind="ExternalInput").ap()
    d_pr = nc.dram_tensor("pr", [HID, 256], f32r, kind="ExternalInput").ap()
    d_out = nc.dram_tensor("out", [BL, F], f32, kind="ExternalOutput").ap()

    # --- SBUF ---
    hTa = nc.alloc_sbuf_tensor("hTa_sb", [HUMAN_DIM + 1, TOK], f32r).ap()
    pa = nc.alloc_sbuf_tensor("pa_sb", [HID, 257], f32).ap()
    pb = nc.alloc_sbuf_tensor("pb_sb", [HID, 320], f32).ap()
    pr_ = nc.alloc_sbuf_tensor("pr_sb", [HID, 256], f32r).ap()
    h1 = nc.alloc_sbuf_tensor("h1_sb", [HID, TOK], f32r).ap()
    h2d = nc.alloc_sbuf_tensor("h2d_sb", [2 * F, BL, NH], f32).ap()
    tmp = nc.alloc_sbuf_tensor("tmp_sb", [2 * F, BL, NH], f32).ap()
    c12 = nc.alloc_sbuf_tensor("c12_sb", [2 * F, BL], f32).ap()
    r1 = nc.alloc_sbuf_tensor("r1_sb", [HID, BL], f32).ap()
    r2 = nc.alloc_sbuf_tensor("r2_sb", [F, BL], f32).ap()
    out_sb = nc.alloc_sbuf_tensor("out_sb", [BL, F], f32).ap()

    # pack_a slices (robot path)
    wr2 = pa[:, 0:64]
    br2 = pa[0:64, 64:65]
    rTa = pa[0:ROBOT_DIM + 1, 65:129]
    wr1a = pa[0:ROBOT_DIM + 1, 129:257]
    # pack_b slices (c-sum + final combine)
    W12 = pb[:, 0:64]
    v12 = pb[:, 64:127]
    bh2d = pb[:, 127:128]
    Ar = pb[0:64, 128:192]
    onesr = pb[0:1, 192:256]
    chebb = pb[0:1, 256:320]
    # packr slices (f32r)
    wh2d = pr_[:, 0:128]
    wh1a = pr_[0:HUMAN_DIM + 1, 128:256]

    # --- PSUM: ping-pong, 8 banks total ---
    ph1 = nc.alloc_psum_tensor("ph1", [HID, 2048], f32).ap()      # 4 banks
    ph2 = nc.alloc_psum_tensor("ph2", [2 * F, 2048], f32).ap()    # 4 banks
    pr1 = ph1[:, 0:BL]          # robot L1 out, freed by r1relu before L1(0)
    pr2 = ph2[:F, 0:BL]         # robot L2 out, freed by r2relu before L2(0)
    po = ph2[:BL, 0:F]          # final out, after relu2(2) consumed ping

    v12_b = bass.AP(v12.tensor, v12.offset, [list(v12.ap[0]), [0, PBATCH], [1, NH]])

    # --- semaphores ---
    sdh = [nc.alloc_semaphore(f"sdh{c}") for c in range(NPAIR)]  # per hTa chunk
    sdr = nc.alloc_semaphore("sdr")    # pack_a DMA
    sdw = nc.alloc_semaphore("sdw")    # packr DMA
    sdf = nc.alloc_semaphore("sdf")    # pack_b DMA
    sp = nc.alloc_semaphore("sp")      # PE groups done
    sa = nc.alloc_semaphore("sa")      # ACT ops done
    sv = nc.alloc_semaphore("sv")      # DVE ops done
    sg = nc.alloc_semaphore("sg")      # GPS ops done
    sq = nc.alloc_semaphore("sq")      # out DMA done
    sdone = nc.alloc_semaphore("sdone")
    all_sems = sdh + [sdr, sdw, sdf, sp, sa, sv, sg, sq, sdone]

    PC = 1008  # tokens per pair

    def ping(p):
        return (p % 2) * 1024

    def pair_b(p):
        return slice(p * PBATCH, (p + 1) * PBATCH)

    def ph_in(ph, p):
        o = ping(p)
        return bass.AP(ph.tensor, ph.offset + o, [list(ph.ap[0]), [512, 2], [1, SL]])

    with nc.Block(no_gpsimd_drain=True) as block:

        @block.sync
        def _(sync):
            for c in range(NPAIR):
                sync.dma_start(
                    out=hTa[:, c * PC : (c + 1) * PC],
                    in_=d_hTa[:, c * PC : (c + 1) * PC],
                ).then_inc(sdh[c], 16)
            sync.wait_ge(sv, 7)
            sync.dma_start(out=d_out[:], in_=out_sb[:]).then_inc(sq, 16)
            sync.wait_ge(sq, 16)
            sync.wait_ge(sdone, 4)

        @block.tensor
        def _(tensor):
            # sp: 1=rMM1 2=rMM2 3=L1(0) 4=L1(1) 5=L2(0) 6=L1(2) 7=L2(1)
            #     8=L1(3) 9=L2(2) 10=L2(3) 11=finals
            def l1(p, *waits):
                for s, v in waits:
                    tensor.wait_ge(s, v)
                o = ping(p)
                tensor.matmul(ph1[:, o : o + SL], wh1a, hTa[:, p * PC : p * PC + SL],
                              start=True, stop=True)
                tensor.matmul(ph1[:, o + 512 : o + 512 + SL], wh1a,
                              hTa[:, p * PC + SL : (p + 1) * PC],
                              start=True, stop=True).then_inc(sp)

            def l2(p, *waits):
                for s, v in waits:
                    tensor.wait_ge(s, v)
                o = ping(p)
                tensor.matmul(ph2[:, o : o + SL], wh2d, h1[:, p * PC : p * PC + SL],
                              start=True, stop=True)
                tensor.matmul(ph2[:, o + 512 : o + 512 + SL], wh2d,
                              h1[:, p * PC + SL : (p + 1) * PC],
                              start=True, stop=True).then_inc(sp)

            tensor.wait_ge(sdr, 16)
            tensor.matmul(pr1, wr1a, rTa, start=True, stop=True).then_inc(sp)   # 1
            tensor.wait_ge(sa, 1)
            tensor.matmul(pr2, wr2, r1[:], start=True, stop=True).then_inc(sp)  # 2
            tensor.wait_ge(sdw, 16)
            l1(0, (sdh[0], 16), (sa, 1))            # 3 (pr1 region WAR)
            l1(1, (sdh[1], 16))                     # 4
            l2(0, (sa, 3))                          # 5 (relu1(0) + r2relu WAR)
            l1(2, (sdh[2], 16), (sa, 3))            # 6 (ping freed)
            l2(1, (sa, 4))                          # 7
            l1(3, (sdh[3], 16), (sa, 4))            # 8
            l2(2, (sa, 6))                          # 9 (ping WAR via relu2(0)<=6)
            l2(3, (sa, 8))                          # 10
            tensor.wait_ge(sv, 6)                   # c12 ready; po region free
            tensor.wait_ge(sdf, 16)
            tensor.matmul(po, c12[:], W12, start=True, stop=False)
            tensor.matmul(po, r2[:], Ar, start=False, stop=False)
            tensor.matmul(po, onesr, chebb, start=False, stop=True).then_inc(sp)  # 11
            tensor.sem_inc(sdone, 1)

        @block.scalar
        def _(scalar):
            scalar.dma_start(out=pa[:], in_=d_pa[:]).then_inc(sdr, 16)
            scalar.dma_start(out=pr_[:], in_=d_pr[:]).then_inc(sdw, 16)
            scalar.dma_start(out=pb[:], in_=d_pb[:]).then_inc(sdf, 16)
            # sa: 1=r1relu 2=r2relu 3=relu1(0) 4=relu1(1) 5=relu2(0)
            #     6=relu1(2) 7=relu2(1) 8=relu1(3)
            scalar.wait_ge(sp, 1)
            scalar.activation(r1[:], pr1, AF.Relu).then_inc(sa)
            scalar.wait_ge(sp, 2)
            scalar.activation(r2[:], pr2, AF.Relu, bias=br2).then_inc(sa)
            scalar.wait_ge(sp, 3)
            scalar.activation(h1[:, 0:PC], ph_in(ph1, 0), AF.Relu).then_inc(sa)
            scalar.wait_ge(sp, 4)
            scalar.activation(h1[:, PC : 2 * PC], ph_in(ph1, 1), AF.Relu).then_inc(sa)
            scalar.wait_ge(sp, 5)
            scalar.wait_ge(sdf, 16)
            scalar.activation(h2d[:, pair_b(0), :], ph_in(ph2, 0), AF.Relu,
                              bias=bh2d).then_inc(sa)
            scalar.wait_ge(sp, 6)
            scalar.activation(h1[:, 2 * PC : 3 * PC], ph_in(ph1, 2), AF.Relu).then_inc(sa)
            scalar.wait_ge(sp, 7)
            scalar.activation(h2d[:, pair_b(1), :], ph_in(ph2, 1), AF.Relu,
                              bias=bh2d).then_inc(sa)
            scalar.wait_ge(sp, 8)
            scalar.activation(h1[:, 3 * PC : 4 * PC], ph_in(ph1, 3), AF.Relu).then_inc(sa)
            scalar.sem_inc(sdone, 1)

        @block.vector
        def _(vector):
            # sv: 1=mul(0) 2=relu2(2) 3=mul(1) 4=relu2(3) 5=red(0) 6=red(1) 7=copy
            vector.wait_ge(sa, 5)
            vector.wait_ge(sdf, 16)
            vector.tensor_tensor(tmp[:, pair_b(0), :], h2d[:, pair_b(0), :],
                                 v12_b, ALU.mult).then_inc(sv)
            vector.wait_ge(sp, 9)
            vector.tensor_scalar(h2d[:, pair_b(2), :], ph_in(ph2, 2), bh2d, 0.0,
                                 op0=ALU.add, op1=ALU.max).then_inc(sv)
            vector.wait_ge(sa, 7)
            vector.tensor_tensor(tmp[:, pair_b(1), :], h2d[:, pair_b(1), :],
                                 v12_b, ALU.mult).then_inc(sv)
            vector.wait_ge(sp, 10)
            vector.tensor_scalar(h2d[:, pair_b(3), :], ph_in(ph2, 3), bh2d, 0.0,
                                 op0=ALU.add, op1=ALU.max).then_inc(sv)
            vector.wait_ge(sv, 3)
            vector.tensor_reduce(c12[:, 0:32], tmp[:, 0:32, :], axis=AX.X,
                                 op=ALU.add).then_inc(sv)
            vector.wait_ge(sg, 2)
            vector.tensor_reduce(c12[:, 32:64], tmp[:, 32:64, :], axis=AX.X,
                                 op=ALU.add).then_inc(sv)
            vector.wait_ge(sp, 11)
            vector.tensor_copy(out_sb[:], po).then_inc(sv)
            vector.sem_inc(sdone, 1)

        @block.gpsimd
        def _(gpsimd):
            gpsimd.wait_ge(sv, 2)
            gpsimd.tensor_tensor(tmp[:, pair_b(2), :], h2d[:, pair_b(2), :],
                                 v12_b, ALU.mult).then_inc(sg)
            gpsimd.wait_ge(sv, 4)
            gpsimd.tensor_tensor(tmp[:, pair_b(3), :], h2d[:, pair_b(3), :],
                                 v12_b, ALU.mult).then_inc(sg)
            gpsimd.sem_inc(sdone, 1)

    # sems must return to 0 for NEFF re-execution; the Block exit emitted an
    # all-engine barrier, so clearing here is safe.
    nc.clear_and_free_semaphores(all_sems)

    nc.compile()
    return nc


def _host_prep(robot_x, human_x, edge_index, wr1_w, wr1_b, wr2_w, wr2_b,
               wh1_w, wh1_b, wh2_w, wh2_b, cheb_w, cheb_b):
    """Compute graph vectors + packed weights on host; build per-core inputs."""
    robot_x = np.ascontiguousarray(np.asarray(robot_x, dtype=np.float32))
    human_x = np.ascontiguousarray(np.asarray(human_x, dtype=np.float32))
    ei = np.asarray(edge_index)
    src, dst = ei[0].astype(np.int64), ei[1].astype(np.int64)

    f32 = np.float32
    deg = np.zeros(N, f32)
    np.add.at(deg, src, f32(1.0))
    dinv = np.where(deg > 0, deg.astype(f32) ** f32(-0.5), f32(0.0)).astype(f32)
    w = -(dinv[src] * dinv[dst])
    L = np.zeros((N, N), f32)
    np.add.at(L, (dst, src), w)
    v1 = L[0].astype(f32)
    v2 = (v1 @ L).astype(f32)

    W0, W1, W2 = (np.asarray(cheb_w, f32)[k] for k in range(3))
    wh1_w = np.asarray(wh1_w, f32); wh1_b = np.asarray(wh1_b, f32)
    wh2_w = np.asarray(wh2_w, f32); wh2_b = np.asarray(wh2_b, f32)
    wr1_w = np.asarray(wr1_w, f32); wr1_b = np.asarray(wr1_b, f32)
    wr2_w = np.asarray(wr2_w, f32); wr2_b = np.asarray(wr2_b, f32)
    cheb_b = np.asarray(cheb_b, f32)

    pa = np.zeros((HID, 257), f32)
    pa[:, 0:64] = wr2_w
    pa[0:64, 64] = wr2_b
    pa[0:ROBOT_DIM, 129:257] = wr1_w
    pa[ROBOT_DIM, 129:257] = wr1_b
    pb = np.zeros((HID, 320), f32)
    pb[0:64, 0:64] = W1
    pb[64:128, 0:64] = f32(2.0) * W2
    pb[0:64, 64:127] = np.tile(v1[1:], (F, 1))
    pb[64:128, 64:127] = np.tile(v2[1:], (F, 1))
    pb[0:64, 127] = wh2_b
    pb[64:128, 127] = wh2_b
    pb[0:64, 128:192] = W0 - W2 + v1[0] * W1 + f32(2.0) * v2[0] * W2
    pb[0, 192:256] = f32(1.0)
    pb[0, 256:320] = cheb_b
    pr = np.zeros((HID, 256), f32)
    pr[:, 0:128] = np.hstack([wh2_w, wh2_w])
    pr[0:HUMAN_DIM, 128:256] = wh1_w
    pr[HUMAN_DIM, 128:256] = wh1_b
    shared = {"pa": pa, "pb": pb, "pr": pr}

    in_maps = []
    ones_tok = np.ones((1, TOK), f32)
    for c in range(NCORES):
        bs = slice(c * BL, (c + 1) * BL)
        hT = human_x[bs].transpose(2, 0, 1).reshape(HUMAN_DIM, TOK)
        m = dict(shared)
        m["hTa"] = np.ascontiguousarray(np.vstack([hT, ones_tok]))
        pac = shared["pa"].copy()
        pac[0:ROBOT_DIM, 65:129] = robot_x[bs, 0, :].T
        pac[ROBOT_DIM, 65:129] = f32(1.0)
        m["pa"] = pac
        in_maps.append(m)
    return in_maps


def run(inputs, trace=False, tmpdir=None):
    """Run the Bass kernel on 8 cores. Returns (full_output, BassKernelResults)."""
    from concourse.bass_utils import run_bass_kernel_spmd

    if "nc" not in _STATE:
        _STATE["nc"] = _build_bass()
    nc = _STATE["nc"]

    in_maps = _host_prep(**inputs)
    res = run_bass_kernel_spmd(
        nc, in_maps, list(range(NCORES)), trace=trace, tmpdir=tmpdir
    )
    out = np.concatenate([res.results[c]["out"] for c in range(NCORES)], axis=0)
    return out, res


def kernel(**inputs) -> np.ndarray:
    out, _ = run(inputs, trace=False)
    return out


# revision 26
# speedup vs baseline: 1.4688x; 1.0514x over previous
"""Trainium2 Bass kernel for nn_DGCRNN (ChebConv K=3 GNN, robot-node output).

Math: the reference returns only node 0 (robot) of the ChebConv output, so
    out = r @ (W0 - W2 + v1[0]*W1 + 2*v2[0]*W2)
        + c1 @ W1 + c2 @ (2*W2) + cheb_b
with v1 = L_hat[0, :], v2 = (L_hat @ L_hat)[0, :] (host-computed from
edge_index), c1 = sum_i v1[i] * h_i, c2 = sum_i v2[i] * h_i over the 63
human-node embeddings h_i, and r the robot embedding.

Sharding: pure data parallel over the batch dim (512 / 8 cores = 64 each);
all weights and graph data replicated.

Implementation: raw bacc (manual semaphores, no Tile) for minimal overhead.
Feature-major layout; MLP layer-1 biases folded into the matmuls via an
appended ones-row (K augmentation); wh2 duplicated along M so h2 lands twice
on 128 partitions, letting one tensor_tensor multiply by the stacked [v1;v2]
pattern and one segmented reduce produce c1,c2 stacked on 128 partitions;
the final ChebConv combine is 3 accumulating matmuls (incl. bias as a rank-1
ones x cheb_b outer product). The big MLP matmuls use float32r (TF32-rate,
single pass); everything else stays fp32.
"""

import numpy as np

B, N, F, HID = 512, 64, 64, 128
ROBOT_DIM, HUMAN_DIM = 9, 5
NCORES = 8
BL = B // NCORES      # 64 batches per core
NH = N - 1            # 63 human nodes
TOK = BL * NH         # 4032 human tokens per core
SL = 504              # tokens per matmul slice (one PSUM bank)
NPAIR = 4             # pipeline pairs; pair = 2 slices = 1008 tokens = 16 batches
PBATCH = 16           # batches per pair

_STATE = {}


def _build_bass():
    import os

    import concourse.bass as bass
    from concourse import bacc, mybir

    f32 = mybir.dt.float32
    f32r = mybir.dt.float32 if os.environ.get("DGCRNN_NO_F32R") else mybir.dt.float32r
    AF = mybir.ActivationFunctionType
    ALU = mybir.AluOpType
    AX = mybir.AxisListType

    nc = bacc.Bacc("TRN2", target_bir_lowering=False, debug=False)

    # --- DRAM I/O ---
    d_hTa = nc.dram_tensor("hTa", [HUMAN_DIM + 1, TOK], f32r, kind="ExternalInput").ap()
    d_pa = nc.dram_tensor("pa", [HID, 257], f32, kind="ExternalInput").ap()
    d_pb = nc.dram_tensor("pb", [HID, 320], f32, kind="ExternalInput").ap()
    d_pr = nc.dram_tensor("pr", [HID, 256], f32r, kind="ExternalInput").ap()
    d_out = nc.dram_tensor("out", [BL, F], f32, kind="ExternalOutput").ap()

    # --- SBUF ---
    hTa = nc.alloc_sbuf_tensor("hTa_sb", [HUMAN_DIM + 1, TOK], f32r).ap()
    pa = nc.alloc_sbuf_tensor("pa_sb", [HID, 257], f32).ap()
    pb = nc.alloc_sbuf_tensor("pb_sb", [HID, 320], f32).ap()
    pr_ = nc.alloc_sbuf_tensor("pr_sb", [HID, 256], f32r).ap()
    h1 = nc.alloc_sbuf_tensor("h1_sb", [HID, TOK], f32r).ap()
    h2d = nc.alloc_sbuf_tensor("h2d_sb", [2 * F, BL, NH], f32).ap()
    tmp = nc.alloc_sbuf_tensor("tmp_sb", [2 * F, BL, NH], f32).ap()
    c12 = nc.alloc_sbuf_tensor("c12_sb", [2 * F, BL], f32).ap()
    r1 = nc.alloc_sbuf_tensor("r1_sb", [HID, BL], f32).ap()
    r2 = nc.alloc_sbuf_tensor("r2_sb", [F, BL], f32).ap()
    out_sb = nc.alloc_sbuf_tensor("out_sb", [BL, F], f32).ap()

    # pack_a slices (robot path)
    wr2 = pa[:, 0:64]
    br2 = pa[0:64, 64:65]
    rTa = pa[0:ROBOT_DIM + 1, 65:129]
    wr1a = pa[0:ROBOT_DIM + 1, 129:257]
    # pack_b slices (c-sum + final combine)
    W12 = pb[:, 0:64]
    v12 = pb[:, 64:127]
    bh2d = pb[:, 127:128]
    Ar = pb[0:64, 128:192]
    onesr = pb[0:1, 192:256]
    chebb = pb[0:1, 256:320]
    # packr slices (f32r)
    wh2d = pr_[:, 0:128]
    wh1a = pr_[0:HUMAN_DIM + 1, 128:256]

    # --- PSUM: ping-pong, 8 banks total ---
    ph1 = nc.alloc_psum_tensor("ph1", [HID, 2048], f32).ap()      # 4 banks
    ph2 = nc.alloc_psum_tensor("ph2", [2 * F, 2048], f32).ap()    # 4 banks
    pr1 = ph1[:, 0:BL]          # robot L1 out, freed by r1relu before L1(0)
    pr2 = ph2[:F, 0:BL]         # robot L2 out, freed by r2relu before L2(0)
    po = ph2[:BL, 0:F]          # final out, after relu2(2) consumed ping

    v12_b = bass.AP(v12.tensor, v12.offset, [list(v12.ap[0]), [0, PBATCH], [1, NH]])

    # --- semaphores ---
    sdh = [nc.alloc_semaphore(f"sdh{c}") for c in range(NPAIR)]  # per hTa chunk
    sdr = nc.alloc_semaphore("sdr")    # pack_a DMA
    sdw = nc.alloc_semaphore("sdw")    # packr DMA
    sdf = nc.alloc_semaphore("sdf")    # pack_b DMA
    sp = nc.alloc_semaphore("sp")      # PE groups done
    sa = nc.alloc_semaphore("sa")      # ACT ops done
    sv = nc.alloc_semaphore("sv")      # DVE ops done
    sg = nc.alloc_semaphore("sg")      # GPS ops done
    # sq is inc-only (out-DMA completion is guaranteed by the end-of-block
    # drain); it is deliberately NOT cleared -- nothing ever waits on it
    sq = nc.alloc_semaphore("sq")
    all_sems = sdh + [sdr, sdw, sdf, sp, sa, sv, sg]

    PC = 1008  # tokens per pair

    def ping(p):
        return (p % 2) * 1024

    def pair_b(p):
        return slice(p * PBATCH, (p + 1) * PBATCH)

    def ph_in(ph, p):
        o = ping(p)
        return bass.AP(ph.tensor, ph.offset + o, [list(ph.ap[0]), [512, 2], [1, SL]])

    with nc.Block(no_gpsimd_drain=True) as block:

        @block.sync
        def _(sync):
            for c in range(NPAIR):
                sync.dma_start(
                    out=hTa[:, c * PC : (c + 1) * PC],
                    in_=d_hTa[:, c * PC : (c + 1) * PC],
                ).then_inc(sdh[c], 16)
            sync.wait_ge(sv, 8)
            sync.dma_start(out=d_out[:], in_=out_sb[:]).then_inc(sq, 16)

        @block.tensor
        def _(tensor):
            # sp: 1=rMM1 2=rMM2 3=L1(0) 4=L1(1) 5=L2(0) 6=L1(2) 7=L2(1)
            #     8=L1(3) 9=L2(2) 10=L2(3) 11=finals
            def l1(p, *waits):
                for s, v in waits:
                    tensor.wait_ge(s, v)
                o = ping(p)
                tensor.matmul(ph1[:, o : o + SL], wh1a, hTa[:, p * PC : p * PC + SL],
                              start=True, stop=True)
                tensor.matmul(ph1[:, o + 512 : o + 512 + SL], wh1a,
                              hTa[:, p * PC + SL : (p + 1) * PC],
                              start=True, stop=True).then_inc(sp)

            def l2(p, *waits):
                for s, v in waits:
                    tensor.wait_ge(s, v)
                o = ping(p)
                tensor.matmul(ph2[:, o : o + SL], wh2d, h1[:, p * PC : p * PC + SL],
                              start=True, stop=True)
                tensor.matmul(ph2[:, o + 512 : o + 512 + SL], wh2d,
                              h1[:, p * PC + SL : (p + 1) * PC],
                              start=True, stop=True).then_inc(sp)

            tensor.wait_ge(sdr, 16)
            tensor.matmul(pr1, wr1a, rTa, start=True, stop=True).then_inc(sp)   # 1
            tensor.wait_ge(sa, 1)
            tensor.matmul(pr2, wr2, r1[:], start=True, stop=True).then_inc(sp)  # 2
            tensor.wait_ge(sdw, 16)
            l1(0, (sdh[0], 16), (sa, 1))            # 3 (pr1 region WAR)
            l1(1, (sdh[1], 16))                     # 4
            l2(0, (sa, 3))                          # 5 (relu1(0) + r2relu WAR)
            l1(2, (sdh[2], 16), (sa, 3))            # 6 (ping freed)
            l2(1, (sa, 4))                          # 7
            l1(3, (sdh[3], 16), (sa, 4))            # 8
            l2(2, (sa, 6))                          # 9 (ping WAR via relu2(0)<=6)
            l2(3, (sa, 8))                          # 10
            tensor.wait_ge(sv, 7)                   # c12 ready
            tensor.wait_ge(sa, 9)                   # po region WAR (relu2(2))
            tensor.wait_ge(sdf, 16)
            tensor.matmul(po, c12[:], W12, start=True, stop=False)
            tensor.matmul(po, r2[:], Ar, start=False, stop=False)
            tensor.matmul(po, onesr, chebb, start=False, stop=True).then_inc(sp)  # 11

        @block.scalar
        def _(scalar):
            scalar.dma_start(out=pa[:], in_=d_pa[:]).then_inc(sdr, 16)
            scalar.dma_start(out=pr_[:], in_=d_pr[:]).then_inc(sdw, 16)
            scalar.dma_start(out=pb[:], in_=d_pb[:]).then_inc(sdf, 16)
            # sa: 1=r1relu 2=r2relu 3=relu1(0) 4=relu1(1) 5=relu2(0)
            #     6=relu1(2) 7=relu2(1) 8=relu1(3)
            scalar.wait_ge(sp, 1)
            scalar.activation(r1[:], pr1, AF.Relu).then_inc(sa)
            scalar.wait_ge(sp, 2)
            scalar.activation(r2[:], pr2, AF.Relu, bias=br2).then_inc(sa)
            scalar.wait_ge(sp, 3)
            scalar.activation(h1[:, 0:PC], ph_in(ph1, 0), AF.Relu).then_inc(sa)
            scalar.wait_ge(sp, 4)
            scalar.activation(h1[:, PC : 2 * PC], ph_in(ph1, 1), AF.Relu).then_inc(sa)
            scalar.wait_ge(sp, 5)
            scalar.wait_ge(sdf, 16)
            scalar.activation(h2d[:, pair_b(0), :], ph_in(ph2, 0), AF.Relu,
                              bias=bh2d).then_inc(sa)
            scalar.wait_ge(sp, 6)
            scalar.activation(h1[:, 2 * PC : 3 * PC], ph_in(ph1, 2), AF.Relu).then_inc(sa)
            scalar.wait_ge(sp, 7)
            scalar.activation(h2d[:, pair_b(1), :], ph_in(ph2, 1), AF.Relu,
                              bias=bh2d).then_inc(sa)
            scalar.wait_ge(sp, 8)
            scalar.activation(h1[:, 3 * PC : 4 * PC], ph_in(ph1, 3), AF.Relu).then_inc(sa)
            scalar.wait_ge(sp, 9)
            scalar.activation(h2d[:, pair_b(2), :], ph_in(ph2, 2), AF.Relu,
                              bias=bh2d).then_inc(sa)
            scalar.wait_ge(sp, 10)
            scalar.activation(h2d[:, pair_b(3), :], ph_in(ph2, 3), AF.Relu,
                              bias=bh2d).then_inc(sa)

        @block.vector
        def _(vector):
            # sv: 1=mul(0) 2=red(0) 3=mul(1) 4=red(1) 5=mul(3) 6=red(3)
            #     7=red(2) 8=copy   (mul(2) on GPS)
            def red(p, *waits):
                for s, v in waits:
                    vector.wait_ge(s, v)
                vector.tensor_reduce(c12[:, pair_b(p)], tmp[:, pair_b(p), :],
                                     axis=AX.X, op=ALU.add).then_inc(sv)

            vector.wait_ge(sa, 5)
            vector.wait_ge(sdf, 16)
            vector.tensor_tensor(tmp[:, pair_b(0), :], h2d[:, pair_b(0), :],
                                 v12_b, ALU.mult).then_inc(sv)
            red(0, (sv, 1))
            vector.wait_ge(sa, 7)
            vector.tensor_tensor(tmp[:, pair_b(1), :], h2d[:, pair_b(1), :],
                                 v12_b, ALU.mult).then_inc(sv)
            red(1, (sv, 3))
            vector.wait_ge(sa, 10)
            vector.tensor_tensor(tmp[:, pair_b(3), :], h2d[:, pair_b(3), :],
                                 v12_b, ALU.mult).then_inc(sv)
            red(3, (sv, 5))
            red(2, (sg, 1))
            vector.wait_ge(sp, 11)
            vector.tensor_copy(out_sb[:], po).then_inc(sv)

        @block.gpsimd
        def _(gpsimd):
            gpsimd.wait_ge(sa, 9)
            gpsimd.tensor_tensor(tmp[:, pair_b(2), :], h2d[:, pair_b(2), :],
                                 v12_b, ALU.mult).then_inc(sg)

    # sems must return to 0 for NEFF re-execution; the Block exit emitted an
    # all-engine barrier, so clearing here is safe.
    nc.clear_and_free_semaphores(all_sems)

    nc.compile()
    return nc


def _host_prep(robot_x, human_x, edge_index, wr1_w, wr1_b, wr2_w, wr2_b,
               wh1_w, wh1_b, wh2_w, wh2_b, cheb_w, cheb_b):
    """Compute graph vectors + packed weights on host; build per-core inputs."""
    robot_x = np.ascontiguousarray(np.asarray(robot_x, dtype=np.float32))
    human_x = np.ascontiguousarray(np.asarray(human_x, dtype=np.float32))
    ei = np.asarray(edge_index)
    src, dst = ei[0].astype(np.int64), ei[1].astype(np.int64)

    f32 = np.float32
    deg = np.zeros(N, f32)
    np.add.at(deg, src, f32(1.0))
    dinv = np.where(deg > 0, deg.astype(f32) ** f32(-0.5), f32(0.0)).astype(f32)
    w = -(dinv[src] * dinv[dst])
    L = np.zeros((N, N), f32)
    np.add.at(L, (dst, src), w)
    v1 = L[0].astype(f32)
    v2 = (v1 @ L).astype(f32)

    W0, W1, W2 = (np.asarray(cheb_w, f32)[k] for k in range(3))
    wh1_w = np.asarray(wh1_w, f32); wh1_b = np.asarray(wh1_b, f32)
    wh2_w = np.asarray(wh2_w, f32); wh2_b = np.asarray(wh2_b, f32)
    wr1_w = np.asarray(wr1_w, f32); wr1_b = np.asarray(wr1_b, f32)
    wr2_w = np.asarray(wr2_w, f32); wr2_b = np.asarray(wr2_b, f32)
    cheb_b = np.asarray(cheb_b, f32)

    pa = np.zeros((HID, 257), f32)
    pa[:, 0:64] = wr2_w
    pa[0:64, 64] = wr2_b
    pa[0:ROBOT_DIM, 129:257] = wr1_w
    pa[ROBOT_DIM, 129:257] = wr1_b
    pb = np.zeros((HID, 320), f32)
    pb[0:64, 0:64] = W1
    pb[64:128, 0:64] = f32(2.0) * W2
    pb[0:64, 64:127] = np.tile(v1[1:], (F, 1))
    pb[64:128, 64:127] = np.tile(v2[1:], (F, 1))
    pb[0:64, 127] = wh2_b
    pb[64:128, 127] = wh2_b
    pb[0:64, 128:192] = W0 - W2 + v1[0] * W1 + f32(2.0) * v2[0] * W2
    pb[0, 192:256] = f32(1.0)
    pb[0, 256:320] = cheb_b
    pr = np.zeros((HID, 256), f32)
    pr[:, 0:128] = np.hstack([wh2_w, wh2_w])
    pr[0:HUMAN_DIM, 128:256] = wh1_w
    pr[HUMAN_DIM, 128:256] = wh1_b
    shared = {"pa": pa, "pb": pb, "pr": pr}

    in_maps = []
    ones_tok = np.ones((1, TOK), f32)
    for c in range(NCORES):
        bs = slice(c * BL, (c + 1) * BL)
        hT = human_x[bs].transpose(2, 0, 1).reshape(HUMAN_DIM, TOK)
        m = dict(shared)
        m["hTa"] = np.ascontiguousarray(np.vstack([hT, ones_tok]))
        pac = shared["pa"].copy()
        pac[0:ROBOT_DIM, 65:129] = robot_x[bs, 0, :].T
        pac[ROBOT_DIM, 65:129] = f32(1.0)
        m["pa"] = pac
        in_maps.append(m)
    return in_maps


def run(inputs, trace=False, tmpdir=None):
    """Run the Bass kernel on 8 cores. Returns (full_output, BassKernelResults)."""
    from concourse.bass_utils import run_bass_kernel_spmd

    if "nc" not in _STATE:
        _STATE["nc"] = _build_bass()
    nc = _STATE["nc"]

    in_maps = _host_prep(**inputs)
    res = run_bass_kernel_spmd(
        nc, in_maps, list(range(NCORES)), trace=trace, tmpdir=tmpdir
    )
    out = np.concatenate([res.results[c]["out"] for c in range(NCORES)], axis=0)
    return out, res


def kernel(**inputs) -> np.ndarray:
    out, _ = run(inputs, trace=False)
    return out
